# revision 60
# baseline (speedup 1.0000x reference)
"""Trainium2 Bass kernel for nn_AttnLayerV3 (differential attention layer).

Strategy: tensor-parallel over heads — 16 heads across 8 NeuronCores
(2 heads/core, each core needs exactly one of the 4 GQA KV heads).
Each core receives x^T plus its weight slices, computes its heads'
attention + per-head GroupNorm + its slice of the output projection,
and returns a partial [T, D] product; the host sums the 8 partials.

Pipeline design (all three phases keep the PE systolic array hot):
 - phase 1 (QKV projections + RoPE): ascending t-strips of 256 with
   double-buffered PSUM rings, so strip s+1's matmuls start while
   strip s's RoPE (Act/DVE/Pool) drains; input DMAs are issued
   fine-grained in compute order so the first matmul starts ~2us in.
 - phase 2 (attention): ascending 512-wide t-strips; the two
   differential-softmax branches (b=0/1) share one PSUM score tile and
   one batched exp per s-block (halves Act instruction overhead); exp
   output (bf16) feeds PV matmuls; softmax combine runs on DVE+Pool
   only, and the [t,ch]->[ch,t] transposes ride the DMA xbar
   (dma_start_transpose on bf16), so the PE stream flows from strip to
   strip with no cross-engine waits. Ascending order in both phases
   means phase 2's first score block depends on phase-1 strips that
   finished long before, erasing the inter-phase seam.
 - phase 3 (output projection): GroupNorm scale is applied per 128-t
   tile just ahead of the matmuls (hides the h1 GN latency); y is
   DMA'd straight from PSUM (no staging copies), per-512 chunks on the
   last tile to shorten the drain.
"""

import math
import os
from contextlib import ExitStack

_KDBG = int(os.environ.get("KDBG", "9"))  # debug: stop after phase N

import numpy as np

import concourse.bacc as bacc
import concourse.tile as tile
from concourse import mybir
from concourse.bass_utils import run_bass_kernel_spmd

f32 = mybir.dt.float32
f32r = mybir.dt.float32r
bf16 = mybir.dt.bfloat16
i32 = mybir.dt.int32
FT = mybir.ActivationFunctionType
ALU = mybir.AluOpType

# problem shape (hardcoded per contract)
B, T, D, H, KV = 1, 2048, 2048, 16, 4
DH = D // H                    # 128
REP = H // KV                  # 4
NCORES = 8
HPC = H // NCORES              # heads per core = 2
CH = HPC * 2 * DH              # 512 output channels per core
LAMBDA_INIT = 0.8 - 0.6 * math.exp(-0.3 * 0)  # 0.2
ROPE_BASE = 10000.0
EPS = 1e-5

SWAP_MASK = [i ^ 1 for i in range(32)]


# ----------------------------------------------------------------------------
# device program
# ----------------------------------------------------------------------------

def _declare_params(nc):
    p = {}
    p["xT"] = nc.dram_tensor("xT", [D, T], bf16, kind="ExternalInput").ap()
    p["wq"] = nc.dram_tensor("wq", [D, CH], bf16, kind="ExternalInput").ap()
    p["wk"] = nc.dram_tensor("wk", [D, 2 * DH], bf16, kind="ExternalInput").ap()
    p["wv"] = nc.dram_tensor("wv", [D, 2 * DH], bf16, kind="ExternalInput").ap()
    p["wo"] = nc.dram_tensor("wo", [CH, D], bf16, kind="ExternalInput").ap()
    p["ropec"] = nc.dram_tensor("ropec", [128, T], f32r, kind="ExternalInput").ap()
    p["ropes"] = nc.dram_tensor("ropes", [128, T], f32r, kind="ExternalInput").ap()
    p["tri"] = nc.dram_tensor("tri", [128, 128], bf16, kind="ExternalInput").ap()
    p["ones"] = nc.dram_tensor("ones", [128, 128], f32r, kind="ExternalInput").ap()
    p["y"] = nc.dram_tensor("y", [T, D], bf16, kind="ExternalOutput").ap()
    return p


def _build(ctx, tc, p, lam):
    nc = tc.nc
    ND = D // 128          # contraction chunks (16)
    NT = T // 128          # t subtiles (16)
    S1 = 256               # phase-1 t-strip width
    NS1 = T // S1          # 8
    SW = 512               # phase-2 t-strip width
    NSW = T // SW          # 4
    NCH = CH // 128        # 4
    NQ = 2 * HPC           # q rows (h-major, [q1,q2] minor) = 4
    scale = 1.0 / math.sqrt(DH)

    # ---------------- constant tiles (resident) ----------------
    consts = ctx.enter_context(tc.tile_pool(name="consts", bufs=1))
    ropec_sb = consts.tile([128, T], f32r, tag="ropec")
    ropes_sb = consts.tile([128, T], f32r, tag="ropes")
    tri_sb = consts.tile([128, 128], bf16, tag="tri")
    ones_sb = consts.tile([128, 128], f32r, tag="ones")
    eps_sb = consts.tile([128, 1], f32, tag="eps")
    nc.vector.memset(eps_sb[:], EPS)
    ones1_sb = consts.tile([128, 1], bf16, tag="ones1")
    nc.vector.memset(ones1_sb[:], 1.0)

    # outT outlives the q/k/v tensors (read by phase 3), so its pool sits
    # below them on the pool stack.
    persist = ctx.enter_context(tc.tile_pool(name="persist", bufs=1))
    outT_sb = persist.tile([128, NCH, T], bf16, tag="outT")      # [ch, chunk, t]

    acts = ctx.enter_context(tc.tile_pool(name="acts", bufs=1))
    qT_sb = acts.tile([128, NQ, T], f32r, tag="qT")              # [dh, (h,b), t]
    kT_sb = acts.tile([128, 2, T], f32r, tag="kT")               # [dh, b, t]
    vaug_sb = acts.tile([128, NT, 2 * DH], bf16, tag="vaug")     # [s, sblock, ch]

    # GroupNorm scratch lives at ctx level: written in phase 2, read in
    # phase 3.
    gpool = ctx.enter_context(tc.tile_pool(name="gn", bufs=2))

    # Phase-2 SBUF pools are allocated BELOW phase 1's stack: a pool opening
    # on a region another pool just released stalls on that region's last
    # readers (pool-granular barrier), and phase 1's rope scratch drains a
    # full RoPE chain after the last matmul.
    epool = ctx.enter_context(tc.tile_pool(name="exp", bufs=20))
    o1pool = ctx.enter_context(tc.tile_pool(name="o1sp", bufs=8))
    cpool = ctx.enter_context(tc.tile_pool(name="comb", bufs=4))
    spool = ctx.enter_context(tc.tile_pool(name="small", bufs=8))

    # ================= phase 1: projections + RoPE =================
    # Drain-first RoPE: all PSUM->SBUF copies go out before any shuffle/mult
    # so the accumulator banks free as fast as Act can copy (the pool-boundary
    # wait into phase 2 is on the LAST PSUM reader).
    def rope_copy(rpool, ps, w, eng=None):
        raw = rpool.tile([128, w], f32r, tag="raw", name="raw", bufs=7)
        if eng is nc.vector:
            nc.vector.tensor_copy(raw[:], ps[:])
        else:
            nc.scalar.copy(raw[:], ps[:])
        return raw

    def rope_finish(rpool, dest, raw, t0, w):
        swp = rpool.tile([128, w], f32, tag="swp", name="swp", bufs=3)
        nc.vector.stream_shuffle(swp[:], raw[:], SWAP_MASK)
        nc.gpsimd.tensor_tensor(dest, raw[:], ropec_sb[:, t0:t0 + w], ALU.mult)
        nc.vector.tensor_tensor(swp[:], swp[:], ropes_sb[:, t0:t0 + w], ALU.mult)
        nc.vector.tensor_tensor(dest, dest, swp[:], ALU.add)

    with ExitStack() as ph1:
        wts = ph1.enter_context(tc.tile_pool(name="wts", bufs=1))
        wq_sb = wts.tile([128, ND, CH], bf16, tag="wq")
        wk_sb = wts.tile([128, ND, 2 * DH], bf16, tag="wk")
        wv_sb = wts.tile([128, ND, 2 * DH], bf16, tag="wv")
        wqr = p["wq"].rearrange("(n p) m -> p n m", p=128)
        wkr = p["wk"].rearrange("(n p) m -> p n m", p=128)
        wvr = p["wv"].rearrange("(n p) m -> p n m", p=128)
        xTr = p["xT"].rearrange("(n p) m -> p n m", p=128)

        DB = 4  # d-chunks per x DMA
        # rpool sits lowest so after ph1 its region is taken by wo (huge DMA
        # slack) rather than the et tiles — rpool's release waits on the last
        # strip's whole RoPE chain.
        rpool = ph1.enter_context(tc.tile_pool(name="rope", bufs=3))
        xpool = ph1.enter_context(tc.tile_pool(name="xt", bufs=5))
        # PSUM is bank-granular: pack two 256-wide accumulators per bank so
        # both strips of a double-buffered pair fit (4 banks/strip x 2).
        p1q = ph1.enter_context(tc.tile_pool(name="p1q", bufs=6, space="PSUM"))
        p1v = ph1.enter_context(tc.tile_pool(name="p1v", bufs=2, space="PSUM"))

        # --- startup DMA schedule: first x chunk, then weights in compute
        # order (fine-grained head, coarse tail), rope tables for strip 0.
        xt0 = xpool.tile([128, DB, S1], bf16, tag="xt", name="xt0")
        nc.sync.dma_start(xt0[:], xTr[:, 0:DB, 0:S1])
        # k matmuls lead each d-group, so wk comes first
        nc.scalar.dma_start(wk_sb[:, 0:4, :], wkr[:, 0:4, :])
        for d in range(4):
            nc.scalar.dma_start(wq_sb[:, d:d + 1, :], wqr[:, d:d + 1, :])
            if d == 0:
                nc.scalar.dma_start(wv_sb[:, 0:4, :], wvr[:, 0:4, :])
        nc.sync.dma_start(ropec_sb[:, 0:S1], p["ropec"][:, 0:S1])
        nc.sync.dma_start(ropes_sb[:, 0:S1], p["ropes"][:, 0:S1])
        for dg in range(1, 4):
            dsl = slice(dg * 4, (dg + 1) * 4)
            nc.scalar.dma_start(wq_sb[:, dsl, :], wqr[:, dsl, :])
            nc.scalar.dma_start(wk_sb[:, dsl, :], wkr[:, dsl, :])
            nc.scalar.dma_start(wv_sb[:, dsl, :], wvr[:, dsl, :])
        nc.scalar.dma_start(tri_sb[:], p["tri"][:])
        nc.scalar.dma_start(ones_sb[:], p["ones"][:])

        for s in range(NS1):
            t0 = s * S1
            psqp = [p1q.tile([128, 2, S1], f32, tag="psq", name=f"psq{_i}")
                    for _i in range(3)]
            psq = [psqp[_m // 2][:, _m % 2, :] for _m in range(NQ + 2)]
            psvp = p1v.tile([128, 2, S1], f32, tag="psv", name="psv")
            psv = [psvp[:, _m, :] for _m in range(S1 // 128)]
            for dg in range(ND // DB):
                if s == 0 and dg == 0:
                    xt = xt0
                else:
                    xt = xpool.tile([128, DB, S1], bf16, tag="xt")
                    nc.sync.dma_start(
                        xt[:], xTr[:, dg * DB:(dg + 1) * DB, t0:t0 + S1])
                if dg == 0 and s + 1 < NS1:  # next strip's rope table chunk
                    t1 = (s + 1) * S1
                    nc.sync.dma_start(ropec_sb[:, t1:t1 + S1],
                                      p["ropec"][:, t1:t1 + S1])
                    nc.sync.dma_start(ropes_sb[:, t1:t1 + S1],
                                      p["ropes"][:, t1:t1 + S1])
                for dq in range(DB):
                    d = dg * DB + dq
                    # PSUM `start` clears the accumulator's whole 2KB bank, so
                    # for bank-sharing pairs only the SECOND slot emits
                    # start=True (first in program order, zeroing both halves)
                    # and its partner accumulates from the cleared bank.
                    korder = (1, 0) if d == 0 else (0, 1)
                    qorder = (1, 0, 3, 2) if d == 0 else (0, 1, 2, 3)
                    vorder = (1, 0) if d == 0 else (0, 1)
                    for j in korder:
                        nc.tensor.matmul(psq[NQ + j][:],
                                         wk_sb[:, d, j * 128:(j + 1) * 128],
                                         xt[:, dq, :],
                                         start=(d == 0 and j % 2 == 1),
                                         stop=(d == ND - 1))
                    for j in qorder:
                        nc.tensor.matmul(psq[j][:], wq_sb[:, d, j * 128:(j + 1) * 128],
                                         xt[:, dq, :],
                                         start=(d == 0 and j % 2 == 1),
                                         stop=(d == ND - 1))
                    for js in vorder:
                        nc.tensor.matmul(psv[js][:], xt[:, dq, js * 128:(js + 1) * 128],
                                         wv_sb[:, d, :],
                                         start=(d == 0 and js % 2 == 1),
                                         stop=(d == ND - 1))
            # last strip: split the PSUM drains across Act and DVE so the
            # pool-boundary wait into phase 2 halves
            last = s == NS1 - 1
            engs = [nc.vector if (last and i % 2 == 1) else nc.scalar
                    for i in range(8)]
            raws = [rope_copy(rpool, psq[NQ + j], S1, engs[j]) for j in range(2)]
            raws += [rope_copy(rpool, psq[j], S1, engs[2 + j]) for j in range(NQ)]
            for js in range(S1 // 128):
                dst = vaug_sb[:, s * (S1 // 128) + js, 0:2 * DH]
                if engs[6 + js] is nc.vector:
                    nc.vector.tensor_copy(dst, psv[js][:])
                else:
                    nc.scalar.copy(dst, psv[js][:])
            for j in range(2):
                rope_finish(rpool, kT_sb[:, j, t0:t0 + S1], raws[j], t0, S1)
            for j in range(NQ):
                rope_finish(rpool, qT_sb[:, j, t0:t0 + S1], raws[2 + j], t0, S1)

    # phase-3 weights: issued here so the DMA overlaps phase 2 (SP queue —
    # the Act queue is saturated by exp in phase 2)
    wop = ctx.enter_context(tc.tile_pool(name="wop", bufs=1))
    wo_sb = wop.tile([128, NCH, D], bf16, tag="wo")
    wor = p["wo"].rearrange("(n p) m -> p n m", p=128)
    for c in range(NCH):
        nc.sync.dma_start(wo_sb[:, c, :], wor[:, c, :])

    if _KDBG < 2:
        with tc.tile_pool(name="dbg", bufs=1) as dbg:
            z = dbg.tile([128, D], bf16, tag="z")
            yrD = p["y"].rearrange("(n p) m -> p n m", p=128)
            for j in range(NQ):     # qT -> tb 0..3
                nc.vector.tensor_copy(z[:], qT_sb[:, j, :])
                nc.sync.dma_start(yrD[:, j, :], z[:])
            for j in range(2):      # kT -> tb 4..5
                nc.vector.tensor_copy(z[:], kT_sb[:, j, :])
                nc.sync.dma_start(yrD[:, 4 + j, :], z[:])
            for g in range(8):      # vaug blocks 0..7 -> tb 8..15
                nc.sync.dma_start(yrD[:, 8 + g, 0:2 * DH],
                                  vaug_sb[:, g, :])
        return

    # ================= phase 2: attention =================
    rstd = {}
    mrs = {}

    def gn_scale(c, tb):
        sl = outT_sb[:, c, tb * 128:(tb + 1) * 128]
        nc.vector.tensor_scalar(sl, sl, rstd[c // 2][:], mrs[c // 2][:],
                                op0=ALU.mult, op1=ALU.add)

    def gn_pre(h, stats):
        """Reduce bn stats for head h down to a [mean_p, E2_p] pair per
        partition; gn_post broadcasts across partitions (ones-matmul into a
        borrowed PSUM slot) and finishes rstd/mrs."""
        mv = gpool.tile([128, 2], f32, tag="mv", name=f"mv{h}")
        nc.vector.bn_aggr(mv[:], stats[:])
        msq = gpool.tile([128, 1], f32, tag="msq", name=f"msq{h}")
        nc.vector.tensor_tensor(msq[:], mv[:, 0:1], mv[:, 0:1], ALU.mult)
        tmp2 = gpool.tile([128, 2], f32r, tag="tmp2", name=f"tmp2{h}")
        nc.vector.tensor_copy(tmp2[:, 0:1], mv[:, 0:1])
        nc.vector.tensor_tensor(tmp2[:, 1:2], mv[:, 1:2], msq[:], ALU.add)
        return tmp2

    def gn_post(h, tmp2, bc):
        nc.tensor.matmul(bc, ones_sb[:], tmp2[:], start=True, stop=True)
        me2 = gpool.tile([128, 2], f32, tag="me2", name=f"me2{h}")
        nc.vector.tensor_scalar_mul(me2[:], bc[:, 0:2], 1.0 / 128)
        m2t = gpool.tile([128, 1], f32, tag="m2t", name=f"m2t{h}")
        nc.vector.tensor_tensor(m2t[:], me2[:, 0:1], me2[:, 0:1], ALU.mult)
        # rstd = rsqrt(var+eps) fully on DVE (magic-constant Newton; 2 iters
        # reach ~4e-6).  Sqrt on the Act engine would force two
        # LoadActFuncSet table swaps that stall the exp pipeline ~2.6us.
        veps = gpool.tile([128, 1], f32, tag="veps", name=f"veps{h}")
        nc.vector.tensor_tensor(veps[:], me2[:, 1:2], m2t[:], ALU.subtract)
        nc.vector.tensor_scalar(veps[:], veps[:], EPS, None, op0=ALU.add)
        ia = gpool.tile([128, 1], i32, tag="ia", name=f"ia{h}")
        ib = gpool.tile([128, 1], i32, tag="ib", name=f"ib{h}")
        nt1 = gpool.tile([128, 1], f32, tag="nt1", name=f"nt1{h}")
        nc.vector.tensor_scalar(ia[:], veps[:].bitcast(i32), 1, None,
                                op0=ALU.arith_shift_right)
        nc.vector.tensor_scalar(ib[:], ia[:], 0x5F3759DF, None,
                                op0=ALU.subtract)
        nc.vector.tensor_scalar(ia[:], ib[:], -1, None, op0=ALU.mult)
        rstd[h] = gpool.tile([128, 1], f32, tag="rstd", name=f"rstd{h}")
        steps = [(ia, ib), (ib, rstd[h])]
        for src, dst in steps:
            y = src[:].bitcast(f32) if src in (ia, ib) else src[:]
            nc.vector.tensor_tensor(nt1[:], y, y, ALU.mult)
            nc.vector.tensor_tensor(nt1[:], nt1[:], veps[:], ALU.mult)
            nc.vector.tensor_scalar(nt1[:], nt1[:], -0.5, 1.5,
                                    op0=ALU.mult, op1=ALU.add)
            d = dst[:].bitcast(f32) if dst in (ia, ib) else dst[:]
            nc.vector.tensor_tensor(d, y, nt1[:], ALU.mult)
        mrs[h] = gpool.tile([128, 1], f32, tag="mrs", name=f"mrs{h}")
        nc.vector.scalar_tensor_tensor(mrs[h][:], me2[:, 0:1], -1.0,
                                       rstd[h][:], ALU.mult, ALU.mult)

    with ExitStack() as ph2:
        scps = ph2.enter_context(tc.tile_pool(name="scps", bufs=2, space="PSUM"))
        ops = ph2.enter_context(tc.tile_pool(name="ops", bufs=4, space="PSUM"))

        def combine(h, jt, kk, o1sp, opb1, d1, d2, stats):
            tb = jt * (SW // 128) + kk
            inv1 = spool.tile([128, 1], f32, tag="inv1", name="inv1")
            nc.vector.reciprocal(inv1[:], d1)
            inv2 = spool.tile([128, 1], f32, tag="inv2", name="inv2")
            nc.vector.reciprocal(inv2[:], d2)
            nlinv2 = spool.tile([128, 1], f32, tag="nlinv2", name="nlinv2")
            nc.vector.tensor_scalar_mul(nlinv2[:], inv2[:], -lam)
            tmp = cpool.tile([128, 2 * DH], f32, tag="tmp", name="tmp")
            nc.vector.tensor_scalar_mul(tmp[:], opb1, nlinv2[:])
            ot = cpool.tile([128, 2 * DH], bf16, tag="ot", name="ot")
            nc.vector.scalar_tensor_tensor(ot[:], o1sp[:], inv1[:],
                                           tmp[:], ALU.mult, ALU.add)
            nc.vector.bn_stats(stats[:, tb, :], ot[:])
            for c in range(2):
                nc.sync.dma_start_transpose(
                    outT_sb[:, h * 2 + c, tb * 128:(tb + 1) * 128],
                    ot[:, c * 128:(c + 1) * 128])

        for h in range(HPC):
            stats = gpool.tile([128, NT, 6], f32, tag="stats", name=f"stats{h}")
            for jt in range(NSW):
                t0 = jt * SW
                nsb = (t0 + SW) // 128
                ets = {}
                # ---- wave A: scores + batched exp + 7 of the 8 PV chains.
                # PSUM banks are paired [b0|b1] per kk (a matmul `start` clears
                # its whole bank, so each bank's second half starts first and
                # its partner accumulates from the cleared bank).  kk3's bank
                # pairs its b0 chain with the 8 softmax-denominator columns
                # (ones-column matmuls).  Act's batched exp and the PE PV
                # stream run fully overlapped this way.
                def tbk(kk):
                    return t0 // 128 + kk
                pair = {kk: ops.tile([128, 2, 2 * DH], f32, tag="op",
                                     name=f"pr{kk}") for kk in range(4)}
                dsum = pair[3][:, 1, :]          # cols 0..7: d[b*4+kk]
                o1 = {}
                for sb in range(nsb):
                    sc2 = scps.tile([128, 2, SW], f32, tag="sc", name="sc2")
                    et2 = epool.tile([128, 2, SW], bf16, tag="et", name="et2")
                    ets[sb] = et2
                    if sb * 128 >= t0:   # diagonal region: trim + tri-mask
                        kd = sb - t0 // 128
                        c0 = min(kd * 128, SW - 256)
                        for b in (0, 1):
                            nc.tensor.matmul(sc2[:, b, c0:],
                                             kT_sb[:, b, sb * 128:(sb + 1) * 128],
                                             qT_sb[:, h * 2 + b, t0 + c0:t0 + SW],
                                             start=True, stop=True)
                        nc.scalar.activation(et2[:, :, c0:], sc2[:, :, c0:],
                                             FT.Exp, scale=scale)
                        for b in (0, 1):
                            nc.gpsimd.tensor_tensor(
                                et2[:, b, kd * 128:(kd + 1) * 128],
                                et2[:, b, kd * 128:(kd + 1) * 128],
                                tri_sb[:], ALU.mult)
                    else:
                        for b in (0, 1):
                            nc.tensor.matmul(sc2[:, b, :],
                                             kT_sb[:, b, sb * 128:(sb + 1) * 128],
                                             qT_sb[:, h * 2 + b, t0:t0 + SW],
                                             start=True, stop=True)
                        nc.scalar.activation(et2[:], sc2[:], FT.Exp, scale=scale)
                    first = sb == 0
                    for kk in range(3):
                        if sb <= tbk(kk):
                            for b in ((1, 0) if first else (0, 1)):
                                nc.tensor.matmul(
                                    pair[kk][:, b, :],
                                    et2[:, b, kk * 128:(kk + 1) * 128],
                                    vaug_sb[:, sb, :],
                                    start=(first and b == 1),
                                    stop=(sb == tbk(kk)))
                    nc.tensor.matmul(pair[3][:, 0, :],
                                     et2[:, 0, 3 * 128:4 * 128],
                                     vaug_sb[:, sb, :],
                                     start=first, stop=(sb == tbk(3)))
                    for b in (0, 1):
                        for kk in range(4):
                            if sb <= tbk(kk):
                                c = b * 4 + kk
                                nc.tensor.matmul(
                                    dsum[:, c:c + 1],
                                    et2[:, b, kk * 128:(kk + 1) * 128],
                                    ones1_sb[:], start=False,
                                    stop=(sb == tbk(kk)))
                    # combines spread inline: each kk's chain just stopped at
                    # sb == tbk(kk), so its softmax combine overlaps the
                    # remaining s-blocks instead of clustering at the tail
                    if t0 // 128 <= sb < t0 // 128 + 3:
                        kk = sb - t0 // 128
                        o1[kk] = o1pool.tile([128, 2 * DH], f32, tag="o1sp",
                                             name=f"o1{kk}")
                        nc.vector.tensor_copy(o1[kk][:], pair[kk][:, 0, :])
                        combine(h, jt, kk, o1[kk], pair[kk][:, 1, :],
                                dsum[:, kk:kk + 1], dsum[:, 4 + kk:5 + kk],
                                stats)
                if h == 1 and jt == NSW - 1:
                    # phase 3's first matmuls need these h0-dependent scales;
                    # emit them ahead of this strip's tail DVE work
                    for tbs in (0, 1):
                        for cs in (0, 1):
                            gn_scale(cs, tbs)
                o1[3] = o1pool.tile([128, 2 * DH], f32, tag="o1sp", name="o13")
                nc.vector.tensor_copy(o1[3][:], pair[3][:, 0, :])
                # ---- wave B: the one remaining chain (b1,kk3) re-reads et ----
                pb = ops.tile([128, 2, 2 * DH], f32, tag="op", name="prB")
                for sb in range(nsb):
                    nc.tensor.matmul(pb[:, 0, :],
                                     ets[sb][:, 1, 3 * 128:4 * 128],
                                     vaug_sb[:, sb, :],
                                     start=(sb == 0), stop=(sb == tbk(3)))
                combine(h, jt, 3, o1[3], pb[:, 0, :],
                        dsum[:, 3:4], dsum[:, 7:8], stats)
                if h == 1 and jt == 0:
                    # h0's GN scalar pipeline: deferred here so its DVE/PE work
                    # hides under h1's first strip
                    bc0 = ops.tile([128, 2, 2 * DH], f32, tag="op",
                                   name="bc0")[:, 0, 0:2]
                    gn_post(0, gn_pre(0, stats0), bc0)
            if h == 0:
                stats0 = stats
            else:
                # h1: only the per-partition reduction here; the broadcast +
                # rstd run inside phase 3 behind its first matmul groups
                tmp2_1 = gn_pre(1, stats)

    if _KDBG < 3:
        with tc.tile_pool(name="dbg", bufs=1) as dbg:
            z = dbg.tile([128, D], bf16, tag="z")
            nc.vector.memset(z[:], 0.0)
            yrD = p["y"].rearrange("(n p) m -> p n m", p=128)
            for tb in range(NT):
                nc.sync.dma_start(yrD[:, tb, :], z[:])
        return

    # ================= phase 3: output projection =================
    with ExitStack() as ph3:
        p3 = ph3.enter_context(tc.tile_pool(name="p3", bufs=2, space="PSUM"))
        ypool = ph3.enter_context(tc.tile_pool(name="yst", bufs=3))
        yr = p["y"].rearrange("(n p) m -> p n m", p=128)

        NS = D // 512

        def mm(py, tb, c, ns, stop):
            nc.tensor.matmul(py[:, ns * 512:(ns + 1) * 512],
                             outT_sb[:, c, tb * 128:(tb + 1) * 128],
                             wo_sb[:, c, ns * 512:(ns + 1) * 512],
                             start=(c == 0), stop=stop)

        def store(py, tb, chunked):
            yst = ypool.tile([128, D], bf16, tag="yst", name="yst")
            for ns in range(NS):
                if ns % 2 == 0:
                    nc.scalar.copy(yst[:, ns * 512:(ns + 1) * 512],
                                   py[:, ns * 512:(ns + 1) * 512])
                else:
                    nc.vector.tensor_copy(yst[:, ns * 512:(ns + 1) * 512],
                                          py[:, ns * 512:(ns + 1) * 512])
                if chunked:   # chunks leave as they land
                    nc.sync.dma_start(yr[:, tb, ns * 512:(ns + 1) * 512],
                                      yst[:, ns * 512:(ns + 1) * 512])
            if not chunked:
                nc.sync.dma_start(yr[:, tb, :], yst[:])

        # --- tiles 0/1: h0 halves first (their scales were pre-issued at the
        # end of phase 2); h1's GN broadcast (PE matmul into py0's last bank)
        # slots in behind them, giving DVE time to finish rstd[1] before the
        # c2/c3 scales are needed.
        pys = {}
        pys[0] = p3.tile([128, D], f32, tag="py", name="py0")
        for c in (0, 1):
            for ns in range(NS - 1):
                mm(pys[0], 0, c, ns, False)
        pys[1] = p3.tile([128, D], f32, tag="py", name="py1")
        for c in (0, 1):
            for ns in range(NS):
                mm(pys[1], 1, c, ns, False)
        gn_post(1, tmp2_1, pys[0][:, (NS - 1) * 512:(NS - 1) * 512 + 2])
        for c in (0, 1):   # py0's last bank starts after bc is consumed
            mm(pys[0], 0, c, NS - 1, False)
        for tb in (0, 1):
            for c in (2, 3):
                gn_scale(c, tb)
                for ns in range(NS):
                    mm(pys[tb], tb, c, ns, c == NCH - 1)
            store(pys[tb], tb, False)
        # --- steady state; last two tiles run ns-major so each 512-chunk's
        # copy+DMA pipelines behind the remaining matmuls
        for tb in range(2, NT):
            py = p3.tile([128, D], f32, tag="py", name="py")
            for c in range(NCH):
                gn_scale(c, tb)
            if tb < NT - 2:
                for c in range(NCH):
                    for ns in range(NS):
                        mm(py, tb, c, ns, c == NCH - 1)
                store(py, tb, False)
            else:
                yst = ypool.tile([128, D], bf16, tag="yst", name="yst")
                for ns in range(NS):
                    for c in range(NCH):
                        mm(py, tb, c, ns, c == NCH - 1)
                    if ns % 2 == 0:
                        nc.scalar.copy(yst[:, ns * 512:(ns + 1) * 512],
                                       py[:, ns * 512:(ns + 1) * 512])
                    else:
                        nc.vector.tensor_copy(yst[:, ns * 512:(ns + 1) * 512],
                                              py[:, ns * 512:(ns + 1) * 512])
                    nc.sync.dma_start(yr[:, tb, ns * 512:(ns + 1) * 512],
                                      yst[:, ns * 512:(ns + 1) * 512])


_prog_cache = {}


def _get_program(lam):
    key = round(float(lam), 9)
    if key in _prog_cache:
        return _prog_cache[key]
    nc = bacc.Bacc("TRN2", target_bir_lowering=False, debug=False)
    p = _declare_params(nc)
    with tile.TileContext(nc) as tc:
        with ExitStack() as ctx:
            _build(ctx, tc, p, lam)
    nc.compile()
    _prog_cache[key] = nc
    return nc


# ----------------------------------------------------------------------------
# host-side input prep
# ----------------------------------------------------------------------------

def _rope_tables():
    inv = 1.0 / (ROPE_BASE ** (np.arange(0, DH, 2, dtype=np.float64) / DH))
    freqs = np.arange(T, dtype=np.float64)[:, None] * inv[None, :]   # [T, 64]
    cos, sin = np.cos(freqs), np.sin(freqs)
    ropec = np.empty((128, T), np.float32)
    ropes = np.empty((128, T), np.float32)
    ropec[0::2, :] = cos.T
    ropec[1::2, :] = cos.T
    ropes[0::2, :] = -sin.T
    ropes[1::2, :] = sin.T
    return ropec, ropes


def _const_inputs():
    import ml_dtypes
    ropec, ropes = _rope_tables()
    tri = (np.arange(128)[:, None] <= np.arange(128)[None, :]).astype(
        ml_dtypes.bfloat16)
    ones = np.ones((128, 128), np.float32)
    return dict(ropec=ropec, ropes=ropes, tri=tri, ones=ones)


def make_in_maps(x, Wq, Wk, Wv, Wo, gn_w):
    import ml_dtypes
    x2d = np.asarray(x, np.float32).reshape(T, D)
    xT = np.ascontiguousarray(x2d.T.astype(ml_dtypes.bfloat16))
    consts = _const_inputs()
    gw = np.asarray(gn_w, np.float64)
    in_maps = []
    for core in range(NCORES):
        h0 = core * HPC
        kv = h0 // REP
        sl = slice(h0 * 2 * DH, (h0 + HPC) * 2 * DH)
        wo = ((1.0 - LAMBDA_INIT) * gw[sl, None]
              * np.asarray(Wo, np.float64)[sl, :]).astype(ml_dtypes.bfloat16)
        in_maps.append(dict(
            xT=xT,
            wq=np.ascontiguousarray(
                np.asarray(Wq, np.float32)[:, sl].astype(ml_dtypes.bfloat16)),
            wk=np.ascontiguousarray(np.asarray(Wk, np.float32)
                [:, kv * 2 * DH:(kv + 1) * 2 * DH].astype(ml_dtypes.bfloat16)),
            wv=np.ascontiguousarray(np.asarray(Wv, np.float32)
                [:, kv * 2 * DH:(kv + 1) * 2 * DH].astype(ml_dtypes.bfloat16)),
            wo=np.ascontiguousarray(wo),
            **consts,
        ))
    return in_maps


def kernel(x, Wq, Wk, Wv, Wo, lambda_q1, lambda_k1, lambda_q2, lambda_k2,
           gn_w, gn_b):
    lam = float(np.exp(np.sum(np.asarray(lambda_q1, np.float64)
                              * np.asarray(lambda_k1, np.float64)))
                - np.exp(np.sum(np.asarray(lambda_q2, np.float64)
                                * np.asarray(lambda_k2, np.float64)))
                + LAMBDA_INIT)
    nc = _get_program(lam)
    in_maps = make_in_maps(x, Wq, Wk, Wv, Wo, gn_w)
    res = run_bass_kernel_spmd(nc, in_maps, list(range(NCORES)))
    y = np.zeros((T, D), np.float64)
    for core in range(NCORES):
        y += res.results[core]["y"].astype(np.float64)
    # gn_b contribution: (1-lambda_init) * gn_b @ Wo, constant over t
    y += (1.0 - LAMBDA_INIT) * (np.asarray(gn_b, np.float64)
                                @ np.asarray(Wo, np.float64))[None, :]
    return y.astype(np.float32).reshape(B, T, D)


# revision 67
# speedup vs baseline: 1.0077x; 1.0077x over previous
"""Trainium2 Bass kernel for nn_AttnLayerV3 (differential attention layer).

Strategy: tensor-parallel over heads — 16 heads across 8 NeuronCores
(2 heads/core, each core needs exactly one of the 4 GQA KV heads).
Each core receives x^T plus its weight slices, computes its heads'
attention + per-head GroupNorm + its slice of the output projection,
and returns a partial [T, D] product; the host sums the 8 partials.

Pipeline design (all three phases keep the PE systolic array hot):
 - phase 1 (QKV projections + RoPE): ascending t-strips of 256 with
   double-buffered PSUM rings, so strip s+1's matmuls start while
   strip s's RoPE (Act/DVE/Pool) drains; input DMAs are issued
   fine-grained in compute order so the first matmul starts ~2us in.
 - phase 2 (attention): ascending 512-wide t-strips; the two
   differential-softmax branches (b=0/1) share one PSUM score tile and
   one batched exp per s-block (halves Act instruction overhead); exp
   output (bf16) feeds PV matmuls; softmax combine runs on DVE+Pool
   only, and the [t,ch]->[ch,t] transposes ride the DMA xbar
   (dma_start_transpose on bf16), so the PE stream flows from strip to
   strip with no cross-engine waits. Ascending order in both phases
   means phase 2's first score block depends on phase-1 strips that
   finished long before, erasing the inter-phase seam.
 - phase 3 (output projection): GroupNorm scale is applied per 128-t
   tile just ahead of the matmuls (hides the h1 GN latency); y is
   DMA'd straight from PSUM (no staging copies), per-512 chunks on the
   last tile to shorten the drain.
"""

import math
import os
from contextlib import ExitStack

_KDBG = int(os.environ.get("KDBG", "9"))  # debug: stop after phase N

import numpy as np

import concourse.bacc as bacc
import concourse.tile as tile
from concourse import mybir
from concourse.bass_utils import run_bass_kernel_spmd

f32 = mybir.dt.float32
f32r = mybir.dt.float32r
bf16 = mybir.dt.bfloat16
i32 = mybir.dt.int32
FT = mybir.ActivationFunctionType
ALU = mybir.AluOpType

# problem shape (hardcoded per contract)
B, T, D, H, KV = 1, 2048, 2048, 16, 4
DH = D // H                    # 128
REP = H // KV                  # 4
NCORES = 8
HPC = H // NCORES              # heads per core = 2
CH = HPC * 2 * DH              # 512 output channels per core
LAMBDA_INIT = 0.8 - 0.6 * math.exp(-0.3 * 0)  # 0.2
ROPE_BASE = 10000.0
EPS = 1e-5

SWAP_MASK = [i ^ 1 for i in range(32)]


# ----------------------------------------------------------------------------
# device program
# ----------------------------------------------------------------------------

def _declare_params(nc):
    p = {}
    p["xT"] = nc.dram_tensor("xT", [D, T], bf16, kind="ExternalInput").ap()
    p["wq"] = nc.dram_tensor("wq", [D, CH], bf16, kind="ExternalInput").ap()
    p["wk"] = nc.dram_tensor("wk", [D, 2 * DH], bf16, kind="ExternalInput").ap()
    p["wv"] = nc.dram_tensor("wv", [D, 2 * DH], bf16, kind="ExternalInput").ap()
    p["wo"] = nc.dram_tensor("wo", [CH, D], bf16, kind="ExternalInput").ap()
    p["ropec"] = nc.dram_tensor("ropec", [128, T], f32r, kind="ExternalInput").ap()
    p["ropes"] = nc.dram_tensor("ropes", [128, T], f32r, kind="ExternalInput").ap()
    p["tri"] = nc.dram_tensor("tri", [128, 128], bf16, kind="ExternalInput").ap()
    p["ones"] = nc.dram_tensor("ones", [128, 128], f32r, kind="ExternalInput").ap()
    p["y"] = nc.dram_tensor("y", [T, D], bf16, kind="ExternalOutput").ap()
    return p


def _build(ctx, tc, p, lam):
    nc = tc.nc
    ND = D // 128          # contraction chunks (16)
    NT = T // 128          # t subtiles (16)
    S1 = 256               # phase-1 t-strip width
    NS1 = T // S1          # 8
    SW = 512               # phase-2 t-strip width
    NSW = T // SW          # 4
    NCH = CH // 128        # 4
    NQ = 2 * HPC           # q rows (h-major, [q1,q2] minor) = 4
    scale = 1.0 / math.sqrt(DH)

    # ---------------- constant tiles (resident) ----------------
    consts = ctx.enter_context(tc.tile_pool(name="consts", bufs=1))
    ropec_sb = consts.tile([128, T], f32r, tag="ropec")
    ropes_sb = consts.tile([128, T], f32r, tag="ropes")
    tri_sb = consts.tile([128, 128], bf16, tag="tri")
    ones_sb = consts.tile([128, 128], f32r, tag="ones")
    eps_sb = consts.tile([128, 1], f32, tag="eps")
    nc.vector.memset(eps_sb[:], EPS)
    ones1_sb = consts.tile([128, 1], bf16, tag="ones1")
    nc.vector.memset(ones1_sb[:], 1.0)

    # outT outlives the q/k/v tensors (read by phase 3), so its pool sits
    # below them on the pool stack.
    persist = ctx.enter_context(tc.tile_pool(name="persist", bufs=1))
    outT_sb = persist.tile([128, NCH, T], bf16, tag="outT")      # [ch, chunk, t]

    acts = ctx.enter_context(tc.tile_pool(name="acts", bufs=1))
    qT_sb = acts.tile([128, NQ, T], f32r, tag="qT")              # [dh, (h,b), t]
    kT_sb = acts.tile([128, 2, T], f32r, tag="kT")               # [dh, b, t]
    vaug_sb = acts.tile([128, NT, 2 * DH], bf16, tag="vaug")     # [s, sblock, ch]

    # GroupNorm scratch lives at ctx level: written in phase 2, read in
    # phase 3.
    gpool = ctx.enter_context(tc.tile_pool(name="gn", bufs=2))

    # Phase-2 SBUF pools are allocated BELOW phase 1's stack: a pool opening
    # on a region another pool just released stalls on that region's last
    # readers (pool-granular barrier), and phase 1's rope scratch drains a
    # full RoPE chain after the last matmul.
    epool = ctx.enter_context(tc.tile_pool(name="exp", bufs=20))
    o1pool = ctx.enter_context(tc.tile_pool(name="o1sp", bufs=8))
    cpool = ctx.enter_context(tc.tile_pool(name="comb", bufs=4))
    spool = ctx.enter_context(tc.tile_pool(name="small", bufs=8))

    # ================= phase 1: projections + RoPE =================
    # Drain-first RoPE: all PSUM->SBUF copies go out before any shuffle/mult
    # so the accumulator banks free as fast as Act can copy (the pool-boundary
    # wait into phase 2 is on the LAST PSUM reader).
    def rope_copy(rpool, ps, w, eng=None):
        raw = rpool.tile([128, w], f32r, tag="raw", name="raw", bufs=7)
        if eng is nc.vector:
            nc.vector.tensor_copy(raw[:], ps[:])
        else:
            nc.scalar.copy(raw[:], ps[:])
        return raw

    def rope_finish(rpool, dest, raw, t0, w):
        swp = rpool.tile([128, w], f32, tag="swp", name="swp", bufs=3)
        nc.vector.stream_shuffle(swp[:], raw[:], SWAP_MASK)
        nc.gpsimd.tensor_tensor(dest, raw[:], ropec_sb[:, t0:t0 + w], ALU.mult)
        nc.vector.tensor_tensor(swp[:], swp[:], ropes_sb[:, t0:t0 + w], ALU.mult)
        nc.vector.tensor_tensor(dest, dest, swp[:], ALU.add)

    with ExitStack() as ph1:
        wts = ph1.enter_context(tc.tile_pool(name="wts", bufs=1))
        wq_sb = wts.tile([128, ND, CH], bf16, tag="wq")
        wk_sb = wts.tile([128, ND, 2 * DH], bf16, tag="wk")
        wv_sb = wts.tile([128, ND, 2 * DH], bf16, tag="wv")
        wqr = p["wq"].rearrange("(n p) m -> p n m", p=128)
        wkr = p["wk"].rearrange("(n p) m -> p n m", p=128)
        wvr = p["wv"].rearrange("(n p) m -> p n m", p=128)
        xTr = p["xT"].rearrange("(n p) m -> p n m", p=128)

        DB = 4  # d-chunks per x DMA
        # rpool sits lowest so after ph1 its region is taken by wo (huge DMA
        # slack) rather than the et tiles — rpool's release waits on the last
        # strip's whole RoPE chain.
        rpool = ph1.enter_context(tc.tile_pool(name="rope", bufs=3))
        xpool = ph1.enter_context(tc.tile_pool(name="xt", bufs=5))
        # PSUM is bank-granular: pack two 256-wide accumulators per bank so
        # both strips of a double-buffered pair fit (4 banks/strip x 2).
        p1q = ph1.enter_context(tc.tile_pool(name="p1q", bufs=6, space="PSUM"))
        p1v = ph1.enter_context(tc.tile_pool(name="p1v", bufs=2, space="PSUM"))

        # --- startup DMA schedule: first x chunk, then weights in compute
        # order (fine-grained head, coarse tail), rope tables for strip 0.
        xt0 = xpool.tile([128, DB, S1], bf16, tag="xt", name="xt0")
        nc.sync.dma_start(xt0[:], xTr[:, 0:DB, 0:S1])
        # k matmuls lead each d-group, so wk comes first
        nc.scalar.dma_start(wk_sb[:, 0:4, :], wkr[:, 0:4, :])
        for d in range(4):
            nc.scalar.dma_start(wq_sb[:, d:d + 1, :], wqr[:, d:d + 1, :])
            if d == 0:
                nc.scalar.dma_start(wv_sb[:, 0:4, :], wvr[:, 0:4, :])
        nc.sync.dma_start(ropec_sb[:, 0:S1], p["ropec"][:, 0:S1])
        nc.sync.dma_start(ropes_sb[:, 0:S1], p["ropes"][:, 0:S1])
        for dg in range(1, 4):
            dsl = slice(dg * 4, (dg + 1) * 4)
            nc.scalar.dma_start(wq_sb[:, dsl, :], wqr[:, dsl, :])
            nc.scalar.dma_start(wk_sb[:, dsl, :], wkr[:, dsl, :])
            nc.scalar.dma_start(wv_sb[:, dsl, :], wvr[:, dsl, :])
        nc.scalar.dma_start(tri_sb[:], p["tri"][:])
        nc.scalar.dma_start(ones_sb[:], p["ones"][:])

        for s in range(NS1):
            t0 = s * S1
            psqp = [p1q.tile([128, 2, S1], f32, tag="psq", name=f"psq{_i}")
                    for _i in range(3)]
            psq = [psqp[_m // 2][:, _m % 2, :] for _m in range(NQ + 2)]
            psvp = p1v.tile([128, 2, S1], f32, tag="psv", name="psv")
            psv = [psvp[:, _m, :] for _m in range(S1 // 128)]
            for dg in range(ND // DB):
                if s == 0 and dg == 0:
                    xt = xt0
                else:
                    xt = xpool.tile([128, DB, S1], bf16, tag="xt")
                    nc.sync.dma_start(
                        xt[:], xTr[:, dg * DB:(dg + 1) * DB, t0:t0 + S1])
                if dg == 0 and s + 1 < NS1:  # next strip's rope table chunk
                    t1 = (s + 1) * S1
                    nc.sync.dma_start(ropec_sb[:, t1:t1 + S1],
                                      p["ropec"][:, t1:t1 + S1])
                    nc.sync.dma_start(ropes_sb[:, t1:t1 + S1],
                                      p["ropes"][:, t1:t1 + S1])
                for dq in range(DB):
                    d = dg * DB + dq
                    # PSUM `start` clears the accumulator's whole 2KB bank, so
                    # for bank-sharing pairs only the SECOND slot emits
                    # start=True (first in program order, zeroing both halves)
                    # and its partner accumulates from the cleared bank.
                    korder = (1, 0) if d == 0 else (0, 1)
                    qorder = (1, 0, 3, 2) if d == 0 else (0, 1, 2, 3)
                    vorder = (1, 0) if d == 0 else (0, 1)
                    for j in korder:
                        nc.tensor.matmul(psq[NQ + j][:],
                                         wk_sb[:, d, j * 128:(j + 1) * 128],
                                         xt[:, dq, :],
                                         start=(d == 0 and j % 2 == 1),
                                         stop=(d == ND - 1))
                    for j in qorder:
                        nc.tensor.matmul(psq[j][:], wq_sb[:, d, j * 128:(j + 1) * 128],
                                         xt[:, dq, :],
                                         start=(d == 0 and j % 2 == 1),
                                         stop=(d == ND - 1))
                    for js in vorder:
                        nc.tensor.matmul(psv[js][:], xt[:, dq, js * 128:(js + 1) * 128],
                                         wv_sb[:, d, :],
                                         start=(d == 0 and js % 2 == 1),
                                         stop=(d == ND - 1))
            # last strip: split the PSUM drains across Act and DVE so the
            # pool-boundary wait into phase 2 halves
            last = s == NS1 - 1
            engs = [nc.vector if (last and i % 2 == 1) else nc.scalar
                    for i in range(8)]
            raws = [rope_copy(rpool, psq[NQ + j], S1, engs[j]) for j in range(2)]
            raws += [rope_copy(rpool, psq[j], S1, engs[2 + j]) for j in range(NQ)]
            for js in range(S1 // 128):
                dst = vaug_sb[:, s * (S1 // 128) + js, 0:2 * DH]
                if engs[6 + js] is nc.vector:
                    nc.vector.tensor_copy(dst, psv[js][:])
                else:
                    nc.scalar.copy(dst, psv[js][:])
            for j in range(2):
                rope_finish(rpool, kT_sb[:, j, t0:t0 + S1], raws[j], t0, S1)
            for j in range(NQ):
                rope_finish(rpool, qT_sb[:, j, t0:t0 + S1], raws[2 + j], t0, S1)

    # phase-3 weights: issued here so the DMA overlaps phase 2 (SP queue —
    # the Act queue is saturated by exp in phase 2)
    wop = ctx.enter_context(tc.tile_pool(name="wop", bufs=1))
    wo_sb = wop.tile([128, NCH, D], bf16, tag="wo")
    wor = p["wo"].rearrange("(n p) m -> p n m", p=128)
    for c in range(NCH):
        nc.sync.dma_start(wo_sb[:, c, :], wor[:, c, :])

    if _KDBG < 2:
        with tc.tile_pool(name="dbg", bufs=1) as dbg:
            z = dbg.tile([128, D], bf16, tag="z")
            yrD = p["y"].rearrange("(n p) m -> p n m", p=128)
            for j in range(NQ):     # qT -> tb 0..3
                nc.vector.tensor_copy(z[:], qT_sb[:, j, :])
                nc.sync.dma_start(yrD[:, j, :], z[:])
            for j in range(2):      # kT -> tb 4..5
                nc.vector.tensor_copy(z[:], kT_sb[:, j, :])
                nc.sync.dma_start(yrD[:, 4 + j, :], z[:])
            for g in range(8):      # vaug blocks 0..7 -> tb 8..15
                nc.sync.dma_start(yrD[:, 8 + g, 0:2 * DH],
                                  vaug_sb[:, g, :])
        return

    # ================= phase 2: attention =================
    rstd = {}
    mrs = {}

    def gn_scale(c, tb, on_act=False):
        sl = outT_sb[:, c, tb * 128:(tb + 1) * 128]
        if on_act:   # Act is idle in phase 3; Identity takes AP scale+bias
            nc.scalar.activation(sl, sl, FT.Identity, bias=mrs[c // 2][:],
                                 scale=rstd[c // 2][:])
        else:
            nc.vector.tensor_scalar(sl, sl, rstd[c // 2][:], mrs[c // 2][:],
                                    op0=ALU.mult, op1=ALU.add)

    def gn_pre(h, stats):
        """Reduce bn stats for head h down to a [mean_p, E2_p] pair per
        partition; gn_post broadcasts across partitions (ones-matmul into a
        borrowed PSUM slot) and finishes rstd/mrs."""
        mv = gpool.tile([128, 2], f32, tag="mv", name=f"mv{h}")
        nc.vector.bn_aggr(mv[:], stats[:])
        msq = gpool.tile([128, 1], f32, tag="msq", name=f"msq{h}")
        nc.vector.tensor_tensor(msq[:], mv[:, 0:1], mv[:, 0:1], ALU.mult)
        tmp2 = gpool.tile([128, 2], f32r, tag="tmp2", name=f"tmp2{h}")
        nc.vector.tensor_copy(tmp2[:, 0:1], mv[:, 0:1])
        nc.vector.tensor_tensor(tmp2[:, 1:2], mv[:, 1:2], msq[:], ALU.add)
        return tmp2

    def gn_post(h, tmp2, bc):
        nc.tensor.matmul(bc, ones_sb[:], tmp2[:], start=True, stop=True)
        me2 = gpool.tile([128, 2], f32, tag="me2", name=f"me2{h}")
        nc.vector.tensor_scalar_mul(me2[:], bc[:, 0:2], 1.0 / 128)
        m2t = gpool.tile([128, 1], f32, tag="m2t", name=f"m2t{h}")
        nc.vector.tensor_tensor(m2t[:], me2[:, 0:1], me2[:, 0:1], ALU.mult)
        # rstd = rsqrt(var+eps) fully on DVE (magic-constant Newton; 2 iters
        # reach ~4e-6).  Sqrt on the Act engine would force two
        # LoadActFuncSet table swaps that stall the exp pipeline ~2.6us.
        veps = gpool.tile([128, 1], f32, tag="veps", name=f"veps{h}")
        nc.vector.tensor_tensor(veps[:], me2[:, 1:2], m2t[:], ALU.subtract)
        nc.vector.tensor_scalar(veps[:], veps[:], EPS, None, op0=ALU.add)
        ia = gpool.tile([128, 1], i32, tag="ia", name=f"ia{h}")
        ib = gpool.tile([128, 1], i32, tag="ib", name=f"ib{h}")
        nt1 = gpool.tile([128, 1], f32, tag="nt1", name=f"nt1{h}")
        nc.vector.tensor_scalar(ia[:], veps[:].bitcast(i32), 1, None,
                                op0=ALU.arith_shift_right)
        nc.vector.tensor_scalar(ib[:], ia[:], 0x5F3759DF, None,
                                op0=ALU.subtract)
        nc.vector.tensor_scalar(ia[:], ib[:], -1, None, op0=ALU.mult)
        rstd[h] = gpool.tile([128, 1], f32, tag="rstd", name=f"rstd{h}")
        steps = [(ia, ib), (ib, rstd[h])]
        for src, dst in steps:
            y = src[:].bitcast(f32) if src in (ia, ib) else src[:]
            nc.vector.tensor_tensor(nt1[:], y, y, ALU.mult)
            nc.vector.tensor_tensor(nt1[:], nt1[:], veps[:], ALU.mult)
            nc.vector.tensor_scalar(nt1[:], nt1[:], -0.5, 1.5,
                                    op0=ALU.mult, op1=ALU.add)
            d = dst[:].bitcast(f32) if dst in (ia, ib) else dst[:]
            nc.vector.tensor_tensor(d, y, nt1[:], ALU.mult)
        mrs[h] = gpool.tile([128, 1], f32, tag="mrs", name=f"mrs{h}")
        nc.vector.scalar_tensor_tensor(mrs[h][:], me2[:, 0:1], -1.0,
                                       rstd[h][:], ALU.mult, ALU.mult)

    with ExitStack() as ph2:
        scps = ph2.enter_context(tc.tile_pool(name="scps", bufs=2, space="PSUM"))
        ops = ph2.enter_context(tc.tile_pool(name="ops", bufs=4, space="PSUM"))

        def combine_pre(d1, d2):
            # reciprocals need only the denominator columns, so they can run
            # as soon as those chains stop — ahead of the PV chain they pair
            # with (shortens the last PSUM read the next pool-open waits on)
            inv1 = spool.tile([128, 1], f32, tag="inv1", name="inv1")
            nc.vector.reciprocal(inv1[:], d1)
            inv2 = spool.tile([128, 1], f32, tag="inv2", name="inv2")
            nc.vector.reciprocal(inv2[:], d2)
            nlinv2 = spool.tile([128, 1], f32, tag="nlinv2", name="nlinv2")
            nc.vector.tensor_scalar_mul(nlinv2[:], inv2[:], -lam)
            return inv1, nlinv2

        def combine_post(h, jt, kk, o1sp, opb1, inv1, nlinv2, stats,
                         act_tmp=False):
            tb = jt * (SW // 128) + kk
            tmp = cpool.tile([128, 2 * DH], f32, tag="tmp", name="tmp")
            if act_tmp:   # Act is idle post-exp; this read is what frees the
                nc.scalar.mul(tmp[:], opb1, nlinv2[:])   # wave-B PSUM bank
            else:
                nc.vector.tensor_scalar_mul(tmp[:], opb1, nlinv2[:])
            ot = cpool.tile([128, 2 * DH], bf16, tag="ot", name="ot")
            nc.vector.scalar_tensor_tensor(ot[:], o1sp[:], inv1[:],
                                           tmp[:], ALU.mult, ALU.add)
            nc.vector.bn_stats(stats[:, tb, :], ot[:])
            for c in range(2):
                nc.sync.dma_start_transpose(
                    outT_sb[:, h * 2 + c, tb * 128:(tb + 1) * 128],
                    ot[:, c * 128:(c + 1) * 128])

        def combine(h, jt, kk, o1sp, opb1, d1, d2, stats):
            inv1, nlinv2 = combine_pre(d1, d2)
            combine_post(h, jt, kk, o1sp, opb1, inv1, nlinv2, stats)

        for h in range(HPC):
            stats = gpool.tile([128, NT, 6], f32, tag="stats", name=f"stats{h}")
            for jt in range(NSW):
                t0 = jt * SW
                nsb = (t0 + SW) // 128
                ets = {}
                # ---- wave A: scores + batched exp + 7 of the 8 PV chains.
                # PSUM banks are paired [b0|b1] per kk (a matmul `start` clears
                # its whole bank, so each bank's second half starts first and
                # its partner accumulates from the cleared bank).  kk3's bank
                # pairs its b0 chain with the 8 softmax-denominator columns
                # (ones-column matmuls).  Act's batched exp and the PE PV
                # stream run fully overlapped this way.
                def tbk(kk):
                    return t0 // 128 + kk
                pair = {kk: ops.tile([128, 2, 2 * DH], f32, tag="op",
                                     name=f"pr{kk}") for kk in range(4)}
                dsum = pair[3][:, 1, :]          # cols 0..7: d[b*4+kk]
                o1 = {}
                for sb in range(nsb):
                    sc2 = scps.tile([128, 2, SW], f32, tag="sc", name="sc2")
                    et2 = epool.tile([128, 2, SW], bf16, tag="et", name="et2")
                    ets[sb] = et2
                    if sb * 128 >= t0:   # diagonal region: trim + tri-mask
                        kd = sb - t0 // 128
                        c0 = min(kd * 128, SW - 256)
                        for b in (0, 1):
                            nc.tensor.matmul(sc2[:, b, c0:],
                                             kT_sb[:, b, sb * 128:(sb + 1) * 128],
                                             qT_sb[:, h * 2 + b, t0 + c0:t0 + SW],
                                             start=True, stop=True)
                        nc.scalar.activation(et2[:, :, c0:], sc2[:, :, c0:],
                                             FT.Exp, scale=scale)
                        for b in (0, 1):
                            nc.gpsimd.tensor_tensor(
                                et2[:, b, kd * 128:(kd + 1) * 128],
                                et2[:, b, kd * 128:(kd + 1) * 128],
                                tri_sb[:], ALU.mult)
                    else:
                        for b in (0, 1):
                            nc.tensor.matmul(sc2[:, b, :],
                                             kT_sb[:, b, sb * 128:(sb + 1) * 128],
                                             qT_sb[:, h * 2 + b, t0:t0 + SW],
                                             start=True, stop=True)
                        nc.scalar.activation(et2[:], sc2[:], FT.Exp, scale=scale)
                    first = sb == 0
                    for kk in range(3):
                        if sb <= tbk(kk):
                            for b in ((1, 0) if first else (0, 1)):
                                nc.tensor.matmul(
                                    pair[kk][:, b, :],
                                    et2[:, b, kk * 128:(kk + 1) * 128],
                                    vaug_sb[:, sb, :],
                                    start=(first and b == 1),
                                    stop=(sb == tbk(kk)))
                    nc.tensor.matmul(pair[3][:, 0, :],
                                     et2[:, 0, 3 * 128:4 * 128],
                                     vaug_sb[:, sb, :],
                                     start=first, stop=(sb == tbk(3)))
                    for b in (0, 1):
                        for kk in range(4):
                            if sb <= tbk(kk):
                                c = b * 4 + kk
                                nc.tensor.matmul(
                                    dsum[:, c:c + 1],
                                    et2[:, b, kk * 128:(kk + 1) * 128],
                                    ones1_sb[:], start=False,
                                    stop=(sb == tbk(kk)))
                    # combines spread inline: each kk's chain just stopped at
                    # sb == tbk(kk), so its softmax combine overlaps the
                    # remaining s-blocks instead of clustering at the tail
                    if t0 // 128 <= sb < t0 // 128 + 3:
                        kk = sb - t0 // 128
                        o1[kk] = o1pool.tile([128, 2 * DH], f32, tag="o1sp",
                                             name=f"o1{kk}")
                        nc.vector.tensor_copy(o1[kk][:], pair[kk][:, 0, :])
                        combine(h, jt, kk, o1[kk], pair[kk][:, 1, :],
                                dsum[:, kk:kk + 1], dsum[:, 4 + kk:5 + kk],
                                stats)
                if h == 1 and jt == NSW - 1:
                    # phase 3's first matmuls need these h0-dependent scales;
                    # emit them ahead of this strip's tail DVE work
                    for tbs in (0, 1):
                        for cs in (0, 1):
                            gn_scale(cs, tbs)
                o1[3] = o1pool.tile([128, 2 * DH], f32, tag="o1sp", name="o13")
                nc.vector.tensor_copy(o1[3][:], pair[3][:, 0, :])
                # kk3's denominators are final here; doing its recips now
                # frees the kk3/dsum bank before wave B even runs
                inv1_3, nlinv2_3 = combine_pre(dsum[:, 3:4], dsum[:, 7:8])
                # ---- wave B: the one remaining chain (b1,kk3) re-reads et ----
                pb = ops.tile([128, 2, 2 * DH], f32, tag="op", name="prB")
                for sb in range(nsb):
                    nc.tensor.matmul(pb[:, 0, :],
                                     ets[sb][:, 1, 3 * 128:4 * 128],
                                     vaug_sb[:, sb, :],
                                     start=(sb == 0), stop=(sb == tbk(3)))
                combine_post(h, jt, 3, o1[3], pb[:, 0, :], inv1_3, nlinv2_3,
                             stats, act_tmp=True)
                if h == 1 and jt == 0:
                    # h0's GN scalar pipeline: deferred here so its DVE/PE work
                    # hides under h1's first strip
                    bc0 = ops.tile([128, 2, 2 * DH], f32, tag="op",
                                   name="bc0")[:, 0, 0:2]
                    gn_post(0, gn_pre(0, stats0), bc0)
            if h == 0:
                stats0 = stats
            else:
                # h1: only the per-partition reduction here; the broadcast +
                # rstd run inside phase 3 behind its first matmul groups
                tmp2_1 = gn_pre(1, stats)

    if _KDBG < 3:
        with tc.tile_pool(name="dbg", bufs=1) as dbg:
            z = dbg.tile([128, D], bf16, tag="z")
            nc.vector.memset(z[:], 0.0)
            yrD = p["y"].rearrange("(n p) m -> p n m", p=128)
            for tb in range(NT):
                nc.sync.dma_start(yrD[:, tb, :], z[:])
        return

    # ================= phase 3: output projection =================
    with ExitStack() as ph3:
        p3 = ph3.enter_context(tc.tile_pool(name="p3", bufs=2, space="PSUM"))
        ypool = ph3.enter_context(tc.tile_pool(name="yst", bufs=3))
        yr = p["y"].rearrange("(n p) m -> p n m", p=128)

        NS = D // 512

        def mm(py, tb, c, ns, stop):
            nc.tensor.matmul(py[:, ns * 512:(ns + 1) * 512],
                             outT_sb[:, c, tb * 128:(tb + 1) * 128],
                             wo_sb[:, c, ns * 512:(ns + 1) * 512],
                             start=(c == 0), stop=stop)

        def store(py, tb, chunked):
            yst = ypool.tile([128, D], bf16, tag="yst", name="yst")
            for ns in range(NS):
                if ns % 2 == 0:
                    nc.scalar.copy(yst[:, ns * 512:(ns + 1) * 512],
                                   py[:, ns * 512:(ns + 1) * 512])
                else:
                    nc.vector.tensor_copy(yst[:, ns * 512:(ns + 1) * 512],
                                          py[:, ns * 512:(ns + 1) * 512])
                if chunked:   # chunks leave as they land
                    nc.sync.dma_start(yr[:, tb, ns * 512:(ns + 1) * 512],
                                      yst[:, ns * 512:(ns + 1) * 512])
            if not chunked:
                nc.sync.dma_start(yr[:, tb, :], yst[:])

        # --- tiles 0/1: h0 halves first (their scales were pre-issued at the
        # end of phase 2); h1's GN broadcast (PE matmul into py0's last bank)
        # slots in behind them, giving DVE time to finish rstd[1] before the
        # c2/c3 scales are needed.
        pys = {}
        pys[0] = p3.tile([128, D], f32, tag="py", name="py0")
        for c in (0, 1):
            for ns in range(NS - 1):
                mm(pys[0], 0, c, ns, False)
        pys[1] = p3.tile([128, D], f32, tag="py", name="py1")
        for c in (0, 1):
            for ns in range(NS):
                mm(pys[1], 1, c, ns, False)
        gn_post(1, tmp2_1, pys[0][:, (NS - 1) * 512:(NS - 1) * 512 + 2])
        for c in (0, 1):   # py0's last bank starts after bc is consumed
            mm(pys[0], 0, c, NS - 1, False)
        for tb in (0, 1):
            for c in (2, 3):
                gn_scale(c, tb)
                for ns in range(NS):
                    mm(pys[tb], tb, c, ns, c == NCH - 1)
            store(pys[tb], tb, False)
        # --- steady state; last two tiles run ns-major so each 512-chunk's
        # copy+DMA pipelines behind the remaining matmuls
        for tb in range(2, NT):
            py = p3.tile([128, D], f32, tag="py", name="py")
            for c in range(NCH):
                gn_scale(c, tb)
            if tb < NT - 2:
                for c in range(NCH):
                    for ns in range(NS):
                        mm(py, tb, c, ns, c == NCH - 1)
                store(py, tb, False)
            else:
                yst = ypool.tile([128, D], bf16, tag="yst", name="yst")
                for ns in range(NS):
                    for c in range(NCH):
                        mm(py, tb, c, ns, c == NCH - 1)
                    if ns % 2 == 0:
                        nc.scalar.copy(yst[:, ns * 512:(ns + 1) * 512],
                                       py[:, ns * 512:(ns + 1) * 512])
                    else:
                        nc.vector.tensor_copy(yst[:, ns * 512:(ns + 1) * 512],
                                              py[:, ns * 512:(ns + 1) * 512])
                    nc.sync.dma_start(yr[:, tb, ns * 512:(ns + 1) * 512],
                                      yst[:, ns * 512:(ns + 1) * 512])


_prog_cache = {}


def _get_program(lam):
    key = round(float(lam), 9)
    if key in _prog_cache:
        return _prog_cache[key]
    nc = bacc.Bacc("TRN2", target_bir_lowering=False, debug=False)
    p = _declare_params(nc)
    with tile.TileContext(nc) as tc:
        with ExitStack() as ctx:
            _build(ctx, tc, p, lam)
    nc.compile()
    _prog_cache[key] = nc
    return nc


# ----------------------------------------------------------------------------
# host-side input prep
# ----------------------------------------------------------------------------

def _rope_tables():
    inv = 1.0 / (ROPE_BASE ** (np.arange(0, DH, 2, dtype=np.float64) / DH))
    freqs = np.arange(T, dtype=np.float64)[:, None] * inv[None, :]   # [T, 64]
    cos, sin = np.cos(freqs), np.sin(freqs)
    ropec = np.empty((128, T), np.float32)
    ropes = np.empty((128, T), np.float32)
    ropec[0::2, :] = cos.T
    ropec[1::2, :] = cos.T
    ropes[0::2, :] = -sin.T
    ropes[1::2, :] = sin.T
    return ropec, ropes


def _const_inputs():
    import ml_dtypes
    ropec, ropes = _rope_tables()
    tri = (np.arange(128)[:, None] <= np.arange(128)[None, :]).astype(
        ml_dtypes.bfloat16)
    ones = np.ones((128, 128), np.float32)
    return dict(ropec=ropec, ropes=ropes, tri=tri, ones=ones)


def make_in_maps(x, Wq, Wk, Wv, Wo, gn_w):
    import ml_dtypes
    x2d = np.asarray(x, np.float32).reshape(T, D)
    xT = np.ascontiguousarray(x2d.T.astype(ml_dtypes.bfloat16))
    consts = _const_inputs()
    gw = np.asarray(gn_w, np.float64)
    in_maps = []
    for core in range(NCORES):
        h0 = core * HPC
        kv = h0 // REP
        sl = slice(h0 * 2 * DH, (h0 + HPC) * 2 * DH)
        wo = ((1.0 - LAMBDA_INIT) * gw[sl, None]
              * np.asarray(Wo, np.float64)[sl, :]).astype(ml_dtypes.bfloat16)
        in_maps.append(dict(
            xT=xT,
            wq=np.ascontiguousarray(
                np.asarray(Wq, np.float32)[:, sl].astype(ml_dtypes.bfloat16)),
            wk=np.ascontiguousarray(np.asarray(Wk, np.float32)
                [:, kv * 2 * DH:(kv + 1) * 2 * DH].astype(ml_dtypes.bfloat16)),
            wv=np.ascontiguousarray(np.asarray(Wv, np.float32)
                [:, kv * 2 * DH:(kv + 1) * 2 * DH].astype(ml_dtypes.bfloat16)),
            wo=np.ascontiguousarray(wo),
            **consts,
        ))
    return in_maps


def kernel(x, Wq, Wk, Wv, Wo, lambda_q1, lambda_k1, lambda_q2, lambda_k2,
           gn_w, gn_b):
    lam = float(np.exp(np.sum(np.asarray(lambda_q1, np.float64)
                              * np.asarray(lambda_k1, np.float64)))
                - np.exp(np.sum(np.asarray(lambda_q2, np.float64)
                                * np.asarray(lambda_k2, np.float64)))
                + LAMBDA_INIT)
    nc = _get_program(lam)
    in_maps = make_in_maps(x, Wq, Wk, Wv, Wo, gn_w)
    res = run_bass_kernel_spmd(nc, in_maps, list(range(NCORES)))
    y = np.zeros((T, D), np.float64)
    for core in range(NCORES):
        y += res.results[core]["y"].astype(np.float64)
    # gn_b contribution: (1-lambda_init) * gn_b @ Wo, constant over t
    y += (1.0 - LAMBDA_INIT) * (np.asarray(gn_b, np.float64)
                                @ np.asarray(Wo, np.float64))[None, :]
    return y.astype(np.float32).reshape(B, T, D)
